# revision 25
# baseline (speedup 1.0000x reference)
"""BitNet-style row-parallel linear on 8 TRN2 NeuronCores.

Reference computes: out[b,s,o] = sum_d x[b,s,d] * sign(w[o,d]) + bias[o]
  x: [4, 2048, 4096] f32, w: [4096, 4096] f32, bias: [4096] f32.

Strategy: data-parallel over the 8192 (b*s) rows — each of the 8 cores
computes a 1024-row slice of the output against the full binarized
weight. No collective needed; shards concatenate to the full output.
(The row-parallel/all-reduce hint costs a 128MB all-reduce per core;
sharding M instead makes the partial outputs disjoint.)

TensorE consumes both operands K-major, so the host preps:
  kxm = x_shard.T           [K=4096, M=1024]  (per core)
  kxn = sign(w).T           [K=4096, N=4096]  (same on every core)
The matmul runs in bf16 (weights are exactly +-1 in bf16; x rounds
to ~1e-3 relative) at 1 PE cycle/row — true fp32 is 4x slower, and
float32r (fp22) costs ~9% more wall time in DMA; see DTYPE below.
"""

import numpy as np

B, S, D_IN, D_OUT = 4, 2048, 4096, 4096
NCORES = 8
M_TOTAL = B * S
M_CORE = M_TOTAL // NCORES

import os

_cache = {}

# "f32r" (fp22 multiply, highest precision) or "bf16" (half the DMA
# traffic + fast weight load; weights are exactly representable).
DTYPE = os.environ.get("BK_DTYPE", "bf16")


IMPL = os.environ.get("BK_IMPL", "lib")


def _custom_body(nc, tc, kxm, kxn, out, mm_dt, mybir):
    """x^T stays SBUF-resident; sign(w)^T streams through once.

    Per n-block of 512 output columns, accumulate k into PSUM banks.
    Block 0 sweeps all 8 banks per k-tile (x still streaming in);
    later blocks run one bank at a time so evictions pipeline and the
    tail after the last matmul is a single evict+store.
    """
    P = 128
    KT = D_IN // P          # 32 k tiles
    MT = M_CORE // P        # 8 m tiles
    NW = 512
    NB = D_OUT // NW        # 8 n blocks
    f32 = mybir.dt.float32

    from contextlib import ExitStack
    with ExitStack() as ctx:
        kxm_pool = ctx.enter_context(tc.tile_pool(name="kxm", bufs=1))
        kxn_pool = ctx.enter_context(tc.tile_pool(name="kxn", bufs=9))
        psum_pool = ctx.enter_context(
            tc.tile_pool(name="psum", bufs=8, space="PSUM"))
        out_pool = ctx.enter_context(tc.tile_pool(name="outp", bufs=8))

        def issue_chunk(nb, c, k0, sz):
            # one kxn chunk: k tiles [k0, k0+sz) of n block nb
            t = kxn_pool.tile([P, sz, NW], mm_dt, tag="kxn",
                              name=f"kxn_{nb}_{c}", bufs=24)
            src = kxn[k0 * P:(k0 + sz) * P, nb * NW:(nb + 1) * NW]
            nc.sync.dma_start(
                out=t, in_=src.rearrange("(ko ki) n -> ki ko n", ki=P))
            return [t[:, i, :] for i in range(sz)]

        def issue_chunks(nb, sizes):
            rhs, k0 = [], 0
            for c, sz in enumerate(sizes):
                rhs += issue_chunk(nb, c, k0, sz)
                k0 += sz
            return rhs

        kxm_tiles = {}

        def issue_kxm(k, h):
            kt = kxm_pool.tile([P, M_CORE // 2], mm_dt, tag="kxm",
                               name=f"kxm_{k}_{h}", bufs=2 * KT)
            eng = nc.scalar if h == 0 else nc.gpsimd
            eng.dma_start(out=kt[:, :],
                          in_=kxm[k * P:(k + 1) * P,
                                  h * (M_CORE // 2):(h + 1) * (M_CORE // 2)])
            kxm_tiles[(k, h)] = kt

        def lhsT(k, m):
            h, off = divmod(m, MT // 2)
            return kxm_tiles[(k, h)][:, off * P:(off + 1) * P]

        # Prologue interleave: x low-halves arrive at sweep-A pace on
        # the scalar queue, weight chunks on sync; x high-halves (for
        # sweep B) trail on the gpsimd queue.
        sizes0 = [2, 2, 2, 2, 4, 4, 4, 4, 4, 4]
        rhs0, k0 = [], 0
        issue_kxm(0, 0)
        issue_kxm(1, 0)
        for c, sz in enumerate(sizes0):
            rhs0 += issue_chunk(0, c, k0, sz)
            k0 += sz
            for k in range(min(k0 + 2, KT)):
                if (k, 0) not in kxm_tiles:
                    issue_kxm(k, 0)
            for k in range(min(k0 - 8, KT)):
                if (k, 1) not in kxm_tiles:
                    issue_kxm(k, 1)
        for k in range(KT):
            if (k, 0) not in kxm_tiles:
                issue_kxm(k, 0)
        for k in range(KT):
            if (k, 1) not in kxm_tiles:
                issue_kxm(k, 1)

        next_rhs = rhs0
        for nb in range(NB):
            ncols = slice(nb * NW, (nb + 1) * NW)
            rhs_k = next_rhs
            psums = [psum_pool.tile([P, NW], f32, tag="ps", name=f"ps_{nb}_{i}")
                     for i in range(MT)]
            # Block 0: two 4-bank sweeps matched to the x-half arrival
            # rate; later blocks: one bank at a time (x resident).
            groups = [range(MT // 2), range(MT // 2, MT)] if nb == 0 \
                else [[m] for m in range(MT)]
            for gi, ms in enumerate(groups):
                for k in range(KT):
                    for m in ms:
                        nc.tensor.matmul(
                            psums[m][:, :],
                            lhsT=lhsT(k, m),
                            rhs=rhs_k[k],
                            start=(k == 0), stop=(k == KT - 1))
                if gi == 0 and nb + 1 < NB:
                    next_rhs = issue_chunks(nb + 1, [4] * 8)
                for m in ms:
                    ot = out_pool.tile([P, NW], f32, tag="ot", name=f"ot_{nb}_{m}")
                    nc.vector.tensor_copy(out=ot[:, :], in_=psums[m][:, :])
                    nc.gpsimd.dma_start(
                        out=out[m * P:(m + 1) * P, ncols], in_=ot[:, :])


def _lib2_body(nc, tc, kxm, kxn, out, mybir):
    """composable_matmul_tile_kernel with two tail optimizations over
    the stock matmul_tile_kernel wrapper: psum eviction on the (idle)
    vector engine instead of ACT, and per-128-row output writes so the
    final post-matmul store is 256KB, not a 1MB m-tile."""
    from contextlib import ExitStack
    from concourse.kernels.tile_matmul import (
        composable_matmul_tile_kernel, dma_from_dram_kxm, dma_from_dram_kxn,
        k_pool_min_bufs)
    from concourse.bass import ds

    with ExitStack() as ctx:
        num_bufs = k_pool_min_bufs(kxn, max_tile_size=512)
        kxm_pool = ctx.enter_context(tc.tile_pool(name="kxm_pool", bufs=num_bufs))
        kxn_pool = ctx.enter_context(tc.tile_pool(name="kxn_pool", bufs=num_bufs))
        kxm_producer, kxm_shape = dma_from_dram_kxm(kxm_pool, kxm)
        kxn_producer, kxn_shape = dma_from_dram_kxn(kxn_pool, kxn)

        out_t = out.rearrange("(po pi) f -> pi po f", pi=128)

        out_eng = os.environ.get("BK_L2_ENG", "gpsimd")

        def mxn_consumer(nc, mxn_tile, md):
            n_sl = ds(md.n_tile_idx * md.n_tile, md.n_slice_size)
            eng = nc.gpsimd if out_eng == "gpsimd" else nc.sync
            for s in range(md.m_subtiles):
                eng.dma_start(
                    out_t[:, ds(md.m_tile_idx * md.m_subtiles + s, 1), n_sl],
                    mxn_tile[:, ds(s, 1), :md.n_slice_size])

        if os.environ.get("BK_L2_RED", "dve") == "dve":
            def reducer(nc, psum, sbuf, md):
                nc.vector.tensor_copy(out=sbuf, in_=psum)
        else:
            from concourse.kernels.tile_matmul import scalar_copyback
            reducer = scalar_copyback()

        composable_matmul_tile_kernel(
            tc=tc,
            kxm_shape=kxm_shape,
            kxn_shape=kxn_shape,
            output_type=mybir.dt.float32,
            kxm_producer=kxm_producer,
            kxn_producer=kxn_producer,
            mxn_consumer=mxn_consumer,
            mxn_subtile_reducer=reducer)


def _lib3_body(nc, tc, kxm, kxn, out, mybir):
    """Stock composable matmul schedule, with one change: x-tile loads
    go through the scalar engine's DMA queue so the weight stream on
    the sync queue is never head-of-line-blocked by them."""
    from contextlib import ExitStack
    from concourse.kernels.tile_matmul import (
        composable_matmul_tile_kernel, dma_from_dram_kxn, dma_to_dram_mxn,
        k_pool_min_bufs, _tiled_ap)
    from concourse.bass import ts, ds

    with ExitStack() as ctx:
        num_bufs = k_pool_min_bufs(kxn, max_tile_size=512)
        kxm_pool = ctx.enter_context(tc.tile_pool(name="kxm_pool", bufs=num_bufs))
        kxn_pool = ctx.enter_context(tc.tile_pool(name="kxn_pool", bufs=num_bufs))

        kxm3, kxm_shape = _tiled_ap(kxm)

        def kxm_producer(nc, md):
            m_slice = min(md.m_tile,
                          kxm_shape.fdims[0] - md.m_tile_idx * md.m_tile)
            t = kxm_pool.tile(
                [128, md.k_subtiles, md.m_tile], kxm.dtype,
                tag=f"kxm_{md.k_subtiles}_{md.m_tile}", name="kxm_t")
            nc.scalar.dma_start(
                t[:, :, :m_slice],
                kxm3[:, ts(md.k_tile_idx, md.k_subtiles),
                     ds(md.m_tile_idx * md.m_tile, m_slice)])
            return t

        kxn_producer, kxn_shape = dma_from_dram_kxn(kxn_pool, kxn)

        composable_matmul_tile_kernel(
            tc=tc,
            kxm_shape=kxm_shape,
            kxn_shape=kxn_shape,
            output_type=mybir.dt.float32,
            kxm_producer=kxm_producer,
            kxn_producer=kxn_producer,
            mxn_consumer=dma_to_dram_mxn(out))


def _build():
    """Build + compile the 8-core SPMD Bass program once per process."""
    if "nc" in _cache:
        return _cache["nc"]

    import concourse.bacc as bacc
    import concourse.tile as tile
    import concourse.mybir as mybir
    from concourse.kernels.tile_matmul import matmul_tile_kernel

    mm_dt = {"f32r": mybir.dt.float32r, "bf16": mybir.dt.bfloat16}[DTYPE]

    nc = bacc.Bacc("TRN2", target_bir_lowering=False, debug=False,
                   enable_asserts=False, num_devices=NCORES)
    kxm = nc.dram_tensor("kxm", [D_IN, M_CORE], mm_dt,
                         kind="ExternalInput").ap()
    kxn = nc.dram_tensor("kxn", [D_IN, D_OUT], mm_dt,
                         kind="ExternalInput").ap()
    out = nc.dram_tensor("out", [M_CORE, D_OUT], mybir.dt.float32,
                         kind="ExternalOutput").ap()
    def _warmup(tc):
        # The PE clock is HAM-throttled to 1.2GHz until ~3.4us of
        # sustained matmul activity. The first real matmul can't start
        # until its DMAs land (~13us in), so burn that window warming
        # the clock gate with matmuls on memset tiles; their PSUM bank
        # frees on pool exit before the real kernel allocates.
        from contextlib import ExitStack
        with ExitStack() as ctx:
            wp = ctx.enter_context(tc.tile_pool(name="warm", bufs=1))
            wpp = ctx.enter_context(
                tc.tile_pool(name="warmp", bufs=1, space="PSUM"))
            wdt = mybir.dt.bfloat16
            a = wp.tile([128, 128], wdt)
            b = wp.tile([128, 512], wdt)
            nc.any.memset(a[:, :], 0.0)
            nc.any.memset(b[:, :], 0.0)
            ps = wpp.tile([128, 512], mybir.dt.float32)
            for _ in range(12):
                nc.tensor.matmul(ps[:, :], lhsT=a[:, :], rhs=b[:, :],
                                 start=True, stop=True)

    if IMPL == "custom":
        with tile.TileContext(nc) as tc:
            _warmup(tc)
            _custom_body(nc, tc, kxm, kxn, out, mm_dt, mybir)
    elif IMPL == "lib3":
        with tile.TileContext(nc) as tc:
            _warmup(tc)
            _lib3_body(nc, tc, kxm, kxn, out, mybir)
    elif IMPL == "lib2":
        with tile.TileContext(nc) as tc:
            _warmup(tc)
            _lib2_body(nc, tc, kxm, kxn, out, mybir)
    else:
        kw = {}
        if os.environ.get("BK_MAX_K_TILE"):
            kw["MAX_K_TILE_SIZE"] = int(os.environ["BK_MAX_K_TILE"])
        if os.environ.get("BK_SKIP_K_SNAKE"):
            kw["skip_k_snake"] = True
        if os.environ.get("BK_NO_CACHE_TILES"):
            kw["cache_tiles"] = False
        with tile.TileContext(nc) as tc:
            _warmup(tc)
            matmul_tile_kernel(tc, kxm, kxn, out, **kw)
    nc.compile()
    _cache["nc"] = nc
    return nc


def _prep_inputs(x, weight):
    if DTYPE == "bf16":
        import ml_dtypes
        np_dt = ml_dtypes.bfloat16
    else:
        np_dt = np.float32
    x2d = np.asarray(x, dtype=np.float32).reshape(M_TOTAL, D_IN)
    kxn = np.ascontiguousarray(np.sign(weight, dtype=np.float32).T.astype(np_dt))
    in_maps = []
    for c in range(NCORES):
        kxm = np.ascontiguousarray(x2d[c * M_CORE:(c + 1) * M_CORE].T.astype(np_dt))
        in_maps.append({"kxm": kxm, "kxn": kxn})
    return in_maps


def _run(x, weight, bias, trace=False):
    from concourse.bass_utils import run_bass_kernel_spmd

    nc = _build()
    in_maps = _prep_inputs(x, weight)
    res = run_bass_kernel_spmd(nc, in_maps, core_ids=list(range(NCORES)),
                               trace=trace)
    out = np.concatenate([res.results[c]["out"] for c in range(NCORES)],
                         axis=0)
    bias = np.asarray(bias, dtype=np.float32)
    if np.any(bias):
        out += bias
    return out.reshape(B, S, D_OUT), res


def kernel(x, weight, bias):
    out, _ = _run(x, weight, bias, trace=False)
    return out


# revision 26
# speedup vs baseline: 1.0138x; 1.0138x over previous
"""BitNet-style row-parallel linear on 8 TRN2 NeuronCores.

Reference computes: out[b,s,o] = sum_d x[b,s,d] * sign(w[o,d]) + bias[o]
  x: [4, 2048, 4096] f32, w: [4096, 4096] f32, bias: [4096] f32.

Strategy: data-parallel over the 8192 (b*s) rows — each of the 8 cores
computes a 1024-row slice of the output against the full binarized
weight. No collective needed; shards concatenate to the full output.
(The row-parallel/all-reduce hint costs a 128MB all-reduce per core;
sharding M instead makes the partial outputs disjoint.)

TensorE consumes both operands K-major, so the host preps:
  kxm = x_shard.T           [K=4096, M=1024]  (per core)
  kxn = sign(w).T           [K=4096, N=4096]  (same on every core)
The matmul runs in bf16 (weights are exactly +-1 in bf16; x rounds
to ~1e-3 relative) at 1 PE cycle/row — true fp32 is 4x slower, and
float32r (fp22) costs ~9% more wall time in DMA; see DTYPE below.
"""

import numpy as np

B, S, D_IN, D_OUT = 4, 2048, 4096, 4096
NCORES = 8
M_TOTAL = B * S
M_CORE = M_TOTAL // NCORES

import os

_cache = {}

# "f32r" (fp22 multiply, highest precision) or "bf16" (half the DMA
# traffic + fast weight load; weights are exactly representable).
DTYPE = os.environ.get("BK_DTYPE", "bf16")


IMPL = os.environ.get("BK_IMPL", "lib")


def _custom_body(nc, tc, kxm, kxn, out, mm_dt, mybir):
    """x^T stays SBUF-resident; sign(w)^T streams through once.

    Per n-block of 512 output columns, accumulate k into PSUM banks.
    Block 0 sweeps all 8 banks per k-tile (x still streaming in);
    later blocks run one bank at a time so evictions pipeline and the
    tail after the last matmul is a single evict+store.
    """
    P = 128
    KT = D_IN // P          # 32 k tiles
    MT = M_CORE // P        # 8 m tiles
    NW = 512
    NB = D_OUT // NW        # 8 n blocks
    f32 = mybir.dt.float32

    from contextlib import ExitStack
    with ExitStack() as ctx:
        kxm_pool = ctx.enter_context(tc.tile_pool(name="kxm", bufs=1))
        kxn_pool = ctx.enter_context(tc.tile_pool(name="kxn", bufs=9))
        psum_pool = ctx.enter_context(
            tc.tile_pool(name="psum", bufs=8, space="PSUM"))
        out_pool = ctx.enter_context(tc.tile_pool(name="outp", bufs=8))

        def issue_chunk(nb, c, k0, sz):
            # one kxn chunk: k tiles [k0, k0+sz) of n block nb
            t = kxn_pool.tile([P, sz, NW], mm_dt, tag="kxn",
                              name=f"kxn_{nb}_{c}", bufs=24)
            src = kxn[k0 * P:(k0 + sz) * P, nb * NW:(nb + 1) * NW]
            nc.sync.dma_start(
                out=t, in_=src.rearrange("(ko ki) n -> ki ko n", ki=P))
            return [t[:, i, :] for i in range(sz)]

        def issue_chunks(nb, sizes):
            rhs, k0 = [], 0
            for c, sz in enumerate(sizes):
                rhs += issue_chunk(nb, c, k0, sz)
                k0 += sz
            return rhs

        kxm_tiles = {}

        def issue_kxm(k, h):
            kt = kxm_pool.tile([P, M_CORE // 2], mm_dt, tag="kxm",
                               name=f"kxm_{k}_{h}", bufs=2 * KT)
            eng = nc.scalar if h == 0 else nc.gpsimd
            eng.dma_start(out=kt[:, :],
                          in_=kxm[k * P:(k + 1) * P,
                                  h * (M_CORE // 2):(h + 1) * (M_CORE // 2)])
            kxm_tiles[(k, h)] = kt

        def lhsT(k, m):
            h, off = divmod(m, MT // 2)
            return kxm_tiles[(k, h)][:, off * P:(off + 1) * P]

        # Prologue interleave: x low-halves arrive at sweep-A pace on
        # the scalar queue, weight chunks on sync; x high-halves (for
        # sweep B) trail on the gpsimd queue.
        sizes0 = [2, 2, 2, 2, 4, 4, 4, 4, 4, 4]
        rhs0, k0 = [], 0
        issue_kxm(0, 0)
        issue_kxm(1, 0)
        for c, sz in enumerate(sizes0):
            rhs0 += issue_chunk(0, c, k0, sz)
            k0 += sz
            for k in range(min(k0 + 2, KT)):
                if (k, 0) not in kxm_tiles:
                    issue_kxm(k, 0)
            for k in range(min(k0 - 8, KT)):
                if (k, 1) not in kxm_tiles:
                    issue_kxm(k, 1)
        for k in range(KT):
            if (k, 0) not in kxm_tiles:
                issue_kxm(k, 0)
        for k in range(KT):
            if (k, 1) not in kxm_tiles:
                issue_kxm(k, 1)

        next_rhs = rhs0
        for nb in range(NB):
            ncols = slice(nb * NW, (nb + 1) * NW)
            rhs_k = next_rhs
            psums = [psum_pool.tile([P, NW], f32, tag="ps", name=f"ps_{nb}_{i}")
                     for i in range(MT)]
            # Block 0: two 4-bank sweeps matched to the x-half arrival
            # rate; later blocks: one bank at a time (x resident).
            groups = [range(MT // 2), range(MT // 2, MT)] if nb == 0 \
                else [[m] for m in range(MT)]
            for gi, ms in enumerate(groups):
                for k in range(KT):
                    for m in ms:
                        nc.tensor.matmul(
                            psums[m][:, :],
                            lhsT=lhsT(k, m),
                            rhs=rhs_k[k],
                            start=(k == 0), stop=(k == KT - 1))
                if gi == 0 and nb + 1 < NB:
                    next_rhs = issue_chunks(nb + 1, [4] * 8)
                for m in ms:
                    ot = out_pool.tile([P, NW], f32, tag="ot", name=f"ot_{nb}_{m}")
                    nc.vector.tensor_copy(out=ot[:, :], in_=psums[m][:, :])
                    nc.gpsimd.dma_start(
                        out=out[m * P:(m + 1) * P, ncols], in_=ot[:, :])


def _build():
    """Build + compile the 8-core SPMD Bass program once per process."""
    if "nc" in _cache:
        return _cache["nc"]

    import concourse.bacc as bacc
    import concourse.tile as tile
    import concourse.mybir as mybir
    from concourse.kernels.tile_matmul import matmul_tile_kernel

    mm_dt = {"f32r": mybir.dt.float32r, "bf16": mybir.dt.bfloat16}[DTYPE]

    nc = bacc.Bacc("TRN2", target_bir_lowering=False, debug=False,
                   enable_asserts=False, num_devices=NCORES)
    kxm = nc.dram_tensor("kxm", [D_IN, M_CORE], mm_dt,
                         kind="ExternalInput").ap()
    kxn = nc.dram_tensor("kxn", [D_IN, D_OUT], mm_dt,
                         kind="ExternalInput").ap()
    out = nc.dram_tensor("out", [M_CORE, D_OUT], mybir.dt.float32,
                         kind="ExternalOutput").ap()
    def _warmup(tc):
        # The PE clock is HAM-throttled to 1.2GHz until ~3.4us of
        # sustained matmul activity. The first real matmul can't start
        # until its DMAs land (~13us in), so burn that window warming
        # the clock gate with matmuls on memset tiles; their PSUM bank
        # frees on pool exit before the real kernel allocates.
        from contextlib import ExitStack
        with ExitStack() as ctx:
            wp = ctx.enter_context(tc.tile_pool(name="warm", bufs=1))
            wpp = ctx.enter_context(
                tc.tile_pool(name="warmp", bufs=1, space="PSUM"))
            wdt = mybir.dt.bfloat16
            a = wp.tile([128, 128], wdt)
            b = wp.tile([128, 512], wdt)
            nc.any.memset(a[:, :], 0.0)
            nc.any.memset(b[:, :], 0.0)
            ps = wpp.tile([128, 512], mybir.dt.float32)
            for _ in range(12):
                nc.tensor.matmul(ps[:, :], lhsT=a[:, :], rhs=b[:, :],
                                 start=True, stop=True)

    if IMPL == "custom":
        with tile.TileContext(nc) as tc:
            _warmup(tc)
            _custom_body(nc, tc, kxm, kxn, out, mm_dt, mybir)
    else:
        kw = {}
        if os.environ.get("BK_MAX_K_TILE"):
            kw["MAX_K_TILE_SIZE"] = int(os.environ["BK_MAX_K_TILE"])
        if os.environ.get("BK_SKIP_K_SNAKE"):
            kw["skip_k_snake"] = True
        if os.environ.get("BK_NO_CACHE_TILES"):
            kw["cache_tiles"] = False
        with tile.TileContext(nc) as tc:
            _warmup(tc)
            matmul_tile_kernel(tc, kxm, kxn, out, **kw)
    nc.compile()
    _cache["nc"] = nc
    return nc


def _prep_inputs(x, weight):
    if DTYPE == "bf16":
        import ml_dtypes
        np_dt = ml_dtypes.bfloat16
    else:
        np_dt = np.float32
    x2d = np.asarray(x, dtype=np.float32).reshape(M_TOTAL, D_IN)
    kxn = np.ascontiguousarray(np.sign(weight, dtype=np.float32).T.astype(np_dt))
    in_maps = []
    for c in range(NCORES):
        kxm = np.ascontiguousarray(x2d[c * M_CORE:(c + 1) * M_CORE].T.astype(np_dt))
        in_maps.append({"kxm": kxm, "kxn": kxn})
    return in_maps


def _run(x, weight, bias, trace=False):
    from concourse.bass_utils import run_bass_kernel_spmd

    nc = _build()
    in_maps = _prep_inputs(x, weight)
    res = run_bass_kernel_spmd(nc, in_maps, core_ids=list(range(NCORES)),
                               trace=trace)
    out = np.concatenate([res.results[c]["out"] for c in range(NCORES)],
                         axis=0)
    bias = np.asarray(bias, dtype=np.float32)
    if np.any(bias):
        out += bias
    return out.reshape(B, S, D_OUT), res


def kernel(x, weight, bias):
    out, _ = _run(x, weight, bias, trace=False)
    return out


# revision 27
# speedup vs baseline: 1.0189x; 1.0050x over previous
"""BitNet-style row-parallel linear on 8 TRN2 NeuronCores.

Reference computes: out[b,s,o] = sum_d x[b,s,d] * sign(w[o,d]) + bias[o]
  x: [4, 2048, 4096] f32, w: [4096, 4096] f32, bias: [4096] f32.

Strategy: data-parallel over the 8192 (b*s) rows — each of the 8 cores
computes a 1024-row slice of the output against the full binarized
weight. No collective needed; shards concatenate to the full output.
(The row-parallel/all-reduce hint costs a 128MB all-reduce per core;
sharding M instead makes the partial outputs disjoint.)

TensorE consumes both operands K-major, so the host preps:
  kxm = x_shard.T           [K=4096, M=1024]  (per core)
  kxn = sign(w).T           [K=4096, N=4096]  (same on every core)
The matmul runs in bf16 (weights are exactly +-1 in bf16; x rounds
to ~1e-3 relative) at 1 PE cycle/row — true fp32 is 4x slower, and
float32r (fp22) costs ~9% more wall time in DMA; see DTYPE below.
"""

import numpy as np

B, S, D_IN, D_OUT = 4, 2048, 4096, 4096
NCORES = 8
M_TOTAL = B * S
M_CORE = M_TOTAL // NCORES

import os

_cache = {}

# "f32r" (fp22 multiply, highest precision) or "bf16" (half the DMA
# traffic + fast weight load; weights are exactly representable).
DTYPE = os.environ.get("BK_DTYPE", "bf16")


IMPL = os.environ.get("BK_IMPL", "lib")


def _custom_body(nc, tc, kxm, kxn, out, mm_dt, mybir):
    """x^T stays SBUF-resident; sign(w)^T streams through once.

    Per n-block of 512 output columns, accumulate k into PSUM banks.
    Block 0 sweeps all 8 banks per k-tile (x still streaming in);
    later blocks run one bank at a time so evictions pipeline and the
    tail after the last matmul is a single evict+store.
    """
    P = 128
    KT = D_IN // P          # 32 k tiles
    MT = M_CORE // P        # 8 m tiles
    NW = 512
    NB = D_OUT // NW        # 8 n blocks
    f32 = mybir.dt.float32

    from contextlib import ExitStack
    with ExitStack() as ctx:
        kxm_pool = ctx.enter_context(tc.tile_pool(name="kxm", bufs=1))
        kxn_pool = ctx.enter_context(tc.tile_pool(name="kxn", bufs=9))
        psum_pool = ctx.enter_context(
            tc.tile_pool(name="psum", bufs=8, space="PSUM"))
        out_pool = ctx.enter_context(tc.tile_pool(name="outp", bufs=8))

        def issue_chunk(nb, c, k0, sz):
            # one kxn chunk: k tiles [k0, k0+sz) of n block nb
            t = kxn_pool.tile([P, sz, NW], mm_dt, tag="kxn",
                              name=f"kxn_{nb}_{c}", bufs=24)
            src = kxn[k0 * P:(k0 + sz) * P, nb * NW:(nb + 1) * NW]
            nc.sync.dma_start(
                out=t, in_=src.rearrange("(ko ki) n -> ki ko n", ki=P))
            return [t[:, i, :] for i in range(sz)]

        def issue_chunks(nb, sizes):
            rhs, k0 = [], 0
            for c, sz in enumerate(sizes):
                rhs += issue_chunk(nb, c, k0, sz)
                k0 += sz
            return rhs

        kxm_tiles = {}

        def issue_kxm(k, h):
            kt = kxm_pool.tile([P, M_CORE // 2], mm_dt, tag="kxm",
                               name=f"kxm_{k}_{h}", bufs=2 * KT)
            eng = nc.scalar if h == 0 else nc.gpsimd
            eng.dma_start(out=kt[:, :],
                          in_=kxm[k * P:(k + 1) * P,
                                  h * (M_CORE // 2):(h + 1) * (M_CORE // 2)])
            kxm_tiles[(k, h)] = kt

        def lhsT(k, m):
            h, off = divmod(m, MT // 2)
            return kxm_tiles[(k, h)][:, off * P:(off + 1) * P]

        # Prologue interleave: x low-halves arrive at sweep-A pace on
        # the scalar queue, weight chunks on sync; x high-halves (for
        # sweep B) trail on the gpsimd queue.
        sizes0 = [2, 2, 2, 2, 4, 4, 4, 4, 4, 4]
        rhs0, k0 = [], 0
        issue_kxm(0, 0)
        issue_kxm(1, 0)
        for c, sz in enumerate(sizes0):
            rhs0 += issue_chunk(0, c, k0, sz)
            k0 += sz
            for k in range(min(k0 + 2, KT)):
                if (k, 0) not in kxm_tiles:
                    issue_kxm(k, 0)
            for k in range(min(k0 - 8, KT)):
                if (k, 1) not in kxm_tiles:
                    issue_kxm(k, 1)
        for k in range(KT):
            if (k, 0) not in kxm_tiles:
                issue_kxm(k, 0)
        for k in range(KT):
            if (k, 1) not in kxm_tiles:
                issue_kxm(k, 1)

        next_rhs = rhs0
        for nb in range(NB):
            ncols = slice(nb * NW, (nb + 1) * NW)
            rhs_k = next_rhs
            psums = [psum_pool.tile([P, NW], f32, tag="ps", name=f"ps_{nb}_{i}")
                     for i in range(MT)]
            # Block 0: two 4-bank sweeps matched to the x-half arrival
            # rate; later blocks: one bank at a time (x resident).
            groups = [range(MT // 2), range(MT // 2, MT)] if nb == 0 \
                else [[m] for m in range(MT)]
            for gi, ms in enumerate(groups):
                for k in range(KT):
                    for m in ms:
                        nc.tensor.matmul(
                            psums[m][:, :],
                            lhsT=lhsT(k, m),
                            rhs=rhs_k[k],
                            start=(k == 0), stop=(k == KT - 1))
                if gi == 0 and nb + 1 < NB:
                    next_rhs = issue_chunks(nb + 1, [4] * 8)
                for m in ms:
                    ot = out_pool.tile([P, NW], f32, tag="ot", name=f"ot_{nb}_{m}")
                    nc.vector.tensor_copy(out=ot[:, :], in_=psums[m][:, :])
                    nc.gpsimd.dma_start(
                        out=out[m * P:(m + 1) * P, ncols], in_=ot[:, :])


def _build():
    """Build + compile the 8-core SPMD Bass program once per process."""
    if "nc" in _cache:
        return _cache["nc"]

    import concourse.bacc as bacc
    import concourse.tile as tile
    import concourse.mybir as mybir
    from concourse.kernels.tile_matmul import matmul_tile_kernel

    mm_dt = {"f32r": mybir.dt.float32r, "bf16": mybir.dt.bfloat16}[DTYPE]

    nc = bacc.Bacc("TRN2", target_bir_lowering=False, debug=False,
                   enable_asserts=False, num_devices=NCORES)
    kxm = nc.dram_tensor("kxm", [D_IN, M_CORE], mm_dt,
                         kind="ExternalInput").ap()
    kxn = nc.dram_tensor("kxn", [D_IN, D_OUT], mm_dt,
                         kind="ExternalInput").ap()
    out = nc.dram_tensor("out", [M_CORE, D_OUT], mybir.dt.float32,
                         kind="ExternalOutput").ap()
    def _warmup(tc):
        # The PE clock is HAM-throttled to 1.2GHz until ~3.4us of
        # sustained matmul activity. The first real matmul can't start
        # until its DMAs land (~13us in), so burn that window warming
        # the clock gate with matmuls on memset tiles; their PSUM bank
        # frees on pool exit before the real kernel allocates.
        from contextlib import ExitStack
        with ExitStack() as ctx:
            wp = ctx.enter_context(tc.tile_pool(name="warm", bufs=1))
            wpp = ctx.enter_context(
                tc.tile_pool(name="warmp", bufs=1, space="PSUM"))
            wdt = mybir.dt.bfloat16
            a = wp.tile([128, 128], wdt)
            b = wp.tile([128, 512], wdt)
            nc.any.memset(a[:, :], 0.0)
            nc.any.memset(b[:, :], 0.0)
            ps = wpp.tile([128, 512], mybir.dt.float32)
            for _ in range(int(os.environ.get("BK_WARM", "12"))):
                nc.tensor.matmul(ps[:, :], lhsT=a[:, :], rhs=b[:, :],
                                 start=True, stop=True)

    if IMPL == "custom":
        with tile.TileContext(nc) as tc:
            _warmup(tc)
            _custom_body(nc, tc, kxm, kxn, out, mm_dt, mybir)
    else:
        kw = {}
        if os.environ.get("BK_MAX_K_TILE"):
            kw["MAX_K_TILE_SIZE"] = int(os.environ["BK_MAX_K_TILE"])
        if os.environ.get("BK_SKIP_K_SNAKE"):
            kw["skip_k_snake"] = True
        if os.environ.get("BK_NO_CACHE_TILES"):
            kw["cache_tiles"] = False
        with tile.TileContext(nc) as tc:
            _warmup(tc)
            matmul_tile_kernel(tc, kxm, kxn, out, **kw)
    nc.compile()
    _cache["nc"] = nc
    return nc


def _prep_inputs(x, weight):
    if DTYPE == "bf16":
        import ml_dtypes
        np_dt = ml_dtypes.bfloat16
    else:
        np_dt = np.float32
    x2d = np.asarray(x, dtype=np.float32).reshape(M_TOTAL, D_IN)
    kxn = np.ascontiguousarray(np.sign(weight, dtype=np.float32).T.astype(np_dt))
    in_maps = []
    for c in range(NCORES):
        kxm = np.ascontiguousarray(x2d[c * M_CORE:(c + 1) * M_CORE].T.astype(np_dt))
        in_maps.append({"kxm": kxm, "kxn": kxn})
    return in_maps


def _run(x, weight, bias, trace=False):
    from concourse.bass_utils import run_bass_kernel_spmd

    nc = _build()
    in_maps = _prep_inputs(x, weight)
    res = run_bass_kernel_spmd(nc, in_maps, core_ids=list(range(NCORES)),
                               trace=trace)
    out = np.concatenate([res.results[c]["out"] for c in range(NCORES)],
                         axis=0)
    bias = np.asarray(bias, dtype=np.float32)
    if np.any(bias):
        out += bias
    return out.reshape(B, S, D_OUT), res


def kernel(x, weight, bias):
    out, _ = _run(x, weight, bias, trace=False)
    return out


# revision 28
# speedup vs baseline: 1.0220x; 1.0031x over previous
"""BitNet-style row-parallel linear on 8 TRN2 NeuronCores.

Reference computes: out[b,s,o] = sum_d x[b,s,d] * sign(w[o,d]) + bias[o]
  x: [4, 2048, 4096] f32, w: [4096, 4096] f32, bias: [4096] f32.

Strategy: data-parallel over the 8192 (b*s) rows — each of the 8 cores
computes a 1024-row slice of the output against the full binarized
weight. No collective needed; shards concatenate to the full output.
(The row-parallel/all-reduce hint costs a 128MB all-reduce per core;
sharding M instead makes the partial outputs disjoint.)

TensorE consumes both operands K-major, so the host preps:
  kxm = x_shard.T           [K=4096, M=1024]  (per core)
  kxn = sign(w).T           [K=4096, N=4096]  (same on every core)
The matmul runs in bf16 (weights are exactly +-1 in bf16; x rounds
to ~1e-3 relative) at 1 PE cycle/row — true fp32 is 4x slower, and
float32r (fp22) costs ~9% more wall time in DMA; see DTYPE below.
"""

import numpy as np

B, S, D_IN, D_OUT = 4, 2048, 4096, 4096
NCORES = 8
M_TOTAL = B * S
M_CORE = M_TOTAL // NCORES

import os

_cache = {}

# "f32r" (fp22 multiply, highest precision) or "bf16" (half the DMA
# traffic + fast weight load; weights are exactly representable).
DTYPE = os.environ.get("BK_DTYPE", "bf16")


IMPL = os.environ.get("BK_IMPL", "lib")


def _custom_body(nc, tc, kxm, kxn, out, mm_dt, mybir):
    """x^T stays SBUF-resident; sign(w)^T streams through once.

    Per n-block of 512 output columns, accumulate k into PSUM banks.
    Block 0 sweeps all 8 banks per k-tile (x still streaming in);
    later blocks run one bank at a time so evictions pipeline and the
    tail after the last matmul is a single evict+store.
    """
    P = 128
    KT = D_IN // P          # 32 k tiles
    MT = M_CORE // P        # 8 m tiles
    NW = 512
    NB = D_OUT // NW        # 8 n blocks
    f32 = mybir.dt.float32

    from contextlib import ExitStack
    with ExitStack() as ctx:
        kxm_pool = ctx.enter_context(tc.tile_pool(name="kxm", bufs=1))
        kxn_pool = ctx.enter_context(tc.tile_pool(name="kxn", bufs=9))
        psum_pool = ctx.enter_context(
            tc.tile_pool(name="psum", bufs=8, space="PSUM"))
        out_pool = ctx.enter_context(tc.tile_pool(name="outp", bufs=8))

        def issue_chunk(nb, c, k0, sz):
            # one kxn chunk: k tiles [k0, k0+sz) of n block nb
            t = kxn_pool.tile([P, sz, NW], mm_dt, tag="kxn",
                              name=f"kxn_{nb}_{c}", bufs=24)
            src = kxn[k0 * P:(k0 + sz) * P, nb * NW:(nb + 1) * NW]
            nc.sync.dma_start(
                out=t, in_=src.rearrange("(ko ki) n -> ki ko n", ki=P))
            return [t[:, i, :] for i in range(sz)]

        def issue_chunks(nb, sizes):
            rhs, k0 = [], 0
            for c, sz in enumerate(sizes):
                rhs += issue_chunk(nb, c, k0, sz)
                k0 += sz
            return rhs

        kxm_tiles = {}

        def issue_kxm(k, h):
            kt = kxm_pool.tile([P, M_CORE // 2], mm_dt, tag="kxm",
                               name=f"kxm_{k}_{h}", bufs=2 * KT)
            eng = nc.scalar if h == 0 else nc.gpsimd
            eng.dma_start(out=kt[:, :],
                          in_=kxm[k * P:(k + 1) * P,
                                  h * (M_CORE // 2):(h + 1) * (M_CORE // 2)])
            kxm_tiles[(k, h)] = kt

        def lhsT(k, m):
            h, off = divmod(m, MT // 2)
            return kxm_tiles[(k, h)][:, off * P:(off + 1) * P]

        # Prologue interleave: x low-halves arrive at sweep-A pace on
        # the scalar queue, weight chunks on sync; x high-halves (for
        # sweep B) trail on the gpsimd queue.
        sizes0 = [2, 2, 2, 2, 4, 4, 4, 4, 4, 4]
        rhs0, k0 = [], 0
        issue_kxm(0, 0)
        issue_kxm(1, 0)
        for c, sz in enumerate(sizes0):
            rhs0 += issue_chunk(0, c, k0, sz)
            k0 += sz
            for k in range(min(k0 + 2, KT)):
                if (k, 0) not in kxm_tiles:
                    issue_kxm(k, 0)
            for k in range(min(k0 - 8, KT)):
                if (k, 1) not in kxm_tiles:
                    issue_kxm(k, 1)
        for k in range(KT):
            if (k, 0) not in kxm_tiles:
                issue_kxm(k, 0)
        for k in range(KT):
            if (k, 1) not in kxm_tiles:
                issue_kxm(k, 1)

        next_rhs = rhs0
        for nb in range(NB):
            ncols = slice(nb * NW, (nb + 1) * NW)
            rhs_k = next_rhs
            psums = [psum_pool.tile([P, NW], f32, tag="ps", name=f"ps_{nb}_{i}")
                     for i in range(MT)]
            # Block 0: two 4-bank sweeps matched to the x-half arrival
            # rate; later blocks: one bank at a time (x resident).
            groups = [range(MT // 2), range(MT // 2, MT)] if nb == 0 \
                else [[m] for m in range(MT)]
            for gi, ms in enumerate(groups):
                for k in range(KT):
                    for m in ms:
                        nc.tensor.matmul(
                            psums[m][:, :],
                            lhsT=lhsT(k, m),
                            rhs=rhs_k[k],
                            start=(k == 0), stop=(k == KT - 1))
                if gi == 0 and nb + 1 < NB:
                    next_rhs = issue_chunks(nb + 1, [4] * 8)
                for m in ms:
                    ot = out_pool.tile([P, NW], f32, tag="ot", name=f"ot_{nb}_{m}")
                    nc.vector.tensor_copy(out=ot[:, :], in_=psums[m][:, :])
                    nc.gpsimd.dma_start(
                        out=out[m * P:(m + 1) * P, ncols], in_=ot[:, :])


def _build():
    """Build + compile the 8-core SPMD Bass program once per process."""
    if "nc" in _cache:
        return _cache["nc"]

    import concourse.bacc as bacc
    import concourse.tile as tile
    import concourse.mybir as mybir
    from concourse.kernels.tile_matmul import matmul_tile_kernel

    mm_dt = {"f32r": mybir.dt.float32r, "bf16": mybir.dt.bfloat16}[DTYPE]

    nc = bacc.Bacc("TRN2", target_bir_lowering=False, debug=False,
                   enable_asserts=bool(os.environ.get("BK_ASSERTS")), num_devices=NCORES)
    kxm = nc.dram_tensor("kxm", [D_IN, M_CORE], mm_dt,
                         kind="ExternalInput").ap()
    kxn = nc.dram_tensor("kxn", [D_IN, D_OUT], mm_dt,
                         kind="ExternalInput").ap()
    out = nc.dram_tensor("out", [M_CORE, D_OUT], mybir.dt.float32,
                         kind="ExternalOutput").ap()
    def _warmup(tc):
        # The PE clock is HAM-throttled to 1.2GHz until ~3.4us of
        # sustained matmul activity. The first real matmul can't start
        # until its DMAs land (~13us in), so burn that window warming
        # the clock gate with matmuls on memset tiles; their PSUM bank
        # frees on pool exit before the real kernel allocates.
        from contextlib import ExitStack
        with ExitStack() as ctx:
            wp = ctx.enter_context(tc.tile_pool(name="warm", bufs=1))
            wpp = ctx.enter_context(
                tc.tile_pool(name="warmp", bufs=1, space="PSUM"))
            wdt = mybir.dt.bfloat16
            a = wp.tile([128, 128], wdt)
            b = wp.tile([128, 512], wdt)
            nc.any.memset(a[:, :], 0.0)
            nc.any.memset(b[:, :], 0.0)
            ps = wpp.tile([128, 512], mybir.dt.float32)
            for _ in range(int(os.environ.get("BK_WARM", "12"))):
                nc.tensor.matmul(ps[:, :], lhsT=a[:, :], rhs=b[:, :],
                                 start=True, stop=True)

    if IMPL == "custom":
        with tile.TileContext(nc) as tc:
            _warmup(tc)
            _custom_body(nc, tc, kxm, kxn, out, mm_dt, mybir)
    else:
        kw = {}
        if os.environ.get("BK_MAX_K_TILE"):
            kw["MAX_K_TILE_SIZE"] = int(os.environ["BK_MAX_K_TILE"])
        if os.environ.get("BK_SKIP_K_SNAKE"):
            kw["skip_k_snake"] = True
        if os.environ.get("BK_NO_CACHE_TILES"):
            kw["cache_tiles"] = False
        with tile.TileContext(nc) as tc:
            _warmup(tc)
            matmul_tile_kernel(tc, kxm, kxn, out, **kw)
    nc.compile()
    _cache["nc"] = nc
    return nc


def _prep_inputs(x, weight):
    if DTYPE == "bf16":
        import ml_dtypes
        np_dt = ml_dtypes.bfloat16
    else:
        np_dt = np.float32
    x2d = np.asarray(x, dtype=np.float32).reshape(M_TOTAL, D_IN)
    kxn = np.ascontiguousarray(np.sign(weight, dtype=np.float32).T.astype(np_dt))
    in_maps = []
    for c in range(NCORES):
        kxm = np.ascontiguousarray(x2d[c * M_CORE:(c + 1) * M_CORE].T.astype(np_dt))
        in_maps.append({"kxm": kxm, "kxn": kxn})
    return in_maps


def _run(x, weight, bias, trace=False):
    from concourse.bass_utils import run_bass_kernel_spmd

    nc = _build()
    in_maps = _prep_inputs(x, weight)
    res = run_bass_kernel_spmd(nc, in_maps, core_ids=list(range(NCORES)),
                               trace=trace)
    out = np.concatenate([res.results[c]["out"] for c in range(NCORES)],
                         axis=0)
    bias = np.asarray(bias, dtype=np.float32)
    if np.any(bias):
        out += bias
    return out.reshape(B, S, D_OUT), res


def kernel(x, weight, bias):
    out, _ = _run(x, weight, bias, trace=False)
    return out


# revision 29
# speedup vs baseline: 1.0232x; 1.0011x over previous
"""BitNet-style row-parallel linear on 8 TRN2 NeuronCores.

Reference computes: out[b,s,o] = sum_d x[b,s,d] * sign(w[o,d]) + bias[o]
  x: [4, 2048, 4096] f32, w: [4096, 4096] f32, bias: [4096] f32.

Strategy: data-parallel over the 8192 (b*s) rows — each of the 8 cores
computes a 1024-row slice of the output against the full binarized
weight. No collective needed; shards concatenate to the full output.
(The row-parallel/all-reduce hint costs a 128MB all-reduce per core;
sharding M instead makes the partial outputs disjoint.)

TensorE consumes both operands K-major, so the host preps:
  kxm = x_shard.T           [K=4096, M=1024]  (per core)
  kxn = sign(w).T           [K=4096, N=4096]  (same on every core)
The matmul runs in bf16 (weights are exactly +-1 in bf16; x rounds
to ~1e-3 relative) at 1 PE cycle/row — true fp32 is 4x slower, and
float32r (fp22) costs ~9% more wall time in DMA; see DTYPE below.
"""

import numpy as np

B, S, D_IN, D_OUT = 4, 2048, 4096, 4096
NCORES = 8
M_TOTAL = B * S
M_CORE = M_TOTAL // NCORES

import os

_cache = {}

# "f32r" (fp22 multiply, highest precision) or "bf16" (half the DMA
# traffic + fast weight load; weights are exactly representable).
DTYPE = os.environ.get("BK_DTYPE", "bf16")


IMPL = os.environ.get("BK_IMPL", "lib")


def _custom_body(nc, tc, kxm, kxn, out, mm_dt, mybir):
    """x^T stays SBUF-resident; sign(w)^T streams through once.

    Per n-block of 512 output columns, accumulate k into PSUM banks.
    Block 0 sweeps all 8 banks per k-tile (x still streaming in);
    later blocks run one bank at a time so evictions pipeline and the
    tail after the last matmul is a single evict+store.
    """
    P = 128
    KT = D_IN // P          # 32 k tiles
    MT = M_CORE // P        # 8 m tiles
    NW = 512
    NB = D_OUT // NW        # 8 n blocks
    f32 = mybir.dt.float32

    from contextlib import ExitStack
    with ExitStack() as ctx:
        kxm_pool = ctx.enter_context(tc.tile_pool(name="kxm", bufs=1))
        kxn_pool = ctx.enter_context(tc.tile_pool(name="kxn", bufs=9))
        psum_pool = ctx.enter_context(
            tc.tile_pool(name="psum", bufs=8, space="PSUM"))
        out_pool = ctx.enter_context(tc.tile_pool(name="outp", bufs=8))

        def issue_chunk(nb, c, k0, sz):
            # one kxn chunk: k tiles [k0, k0+sz) of n block nb
            t = kxn_pool.tile([P, sz, NW], mm_dt, tag="kxn",
                              name=f"kxn_{nb}_{c}", bufs=24)
            src = kxn[k0 * P:(k0 + sz) * P, nb * NW:(nb + 1) * NW]
            nc.sync.dma_start(
                out=t, in_=src.rearrange("(ko ki) n -> ki ko n", ki=P))
            return [t[:, i, :] for i in range(sz)]

        def issue_chunks(nb, sizes):
            rhs, k0 = [], 0
            for c, sz in enumerate(sizes):
                rhs += issue_chunk(nb, c, k0, sz)
                k0 += sz
            return rhs

        kxm_tiles = {}

        def issue_kxm(k, h):
            kt = kxm_pool.tile([P, M_CORE // 2], mm_dt, tag="kxm",
                               name=f"kxm_{k}_{h}", bufs=2 * KT)
            eng = nc.scalar if h == 0 else nc.gpsimd
            eng.dma_start(out=kt[:, :],
                          in_=kxm[k * P:(k + 1) * P,
                                  h * (M_CORE // 2):(h + 1) * (M_CORE // 2)])
            kxm_tiles[(k, h)] = kt

        def lhsT(k, m):
            h, off = divmod(m, MT // 2)
            return kxm_tiles[(k, h)][:, off * P:(off + 1) * P]

        # Prologue interleave: x low-halves arrive at sweep-A pace on
        # the scalar queue, weight chunks on sync; x high-halves (for
        # sweep B) trail on the gpsimd queue.
        sizes0 = [2, 2, 2, 2, 4, 4, 4, 4, 4, 4]
        rhs0, k0 = [], 0
        issue_kxm(0, 0)
        issue_kxm(1, 0)
        for c, sz in enumerate(sizes0):
            rhs0 += issue_chunk(0, c, k0, sz)
            k0 += sz
            for k in range(min(k0 + 2, KT)):
                if (k, 0) not in kxm_tiles:
                    issue_kxm(k, 0)
            for k in range(min(k0 - 8, KT)):
                if (k, 1) not in kxm_tiles:
                    issue_kxm(k, 1)
        for k in range(KT):
            if (k, 0) not in kxm_tiles:
                issue_kxm(k, 0)
        for k in range(KT):
            if (k, 1) not in kxm_tiles:
                issue_kxm(k, 1)

        next_rhs = rhs0
        for nb in range(NB):
            ncols = slice(nb * NW, (nb + 1) * NW)
            rhs_k = next_rhs
            psums = [psum_pool.tile([P, NW], f32, tag="ps", name=f"ps_{nb}_{i}")
                     for i in range(MT)]
            # Block 0: two 4-bank sweeps matched to the x-half arrival
            # rate; later blocks: one bank at a time (x resident).
            groups = [range(MT // 2), range(MT // 2, MT)] if nb == 0 \
                else [[m] for m in range(MT)]
            for gi, ms in enumerate(groups):
                for k in range(KT):
                    for m in ms:
                        nc.tensor.matmul(
                            psums[m][:, :],
                            lhsT=lhsT(k, m),
                            rhs=rhs_k[k],
                            start=(k == 0), stop=(k == KT - 1))
                if gi == 0 and nb + 1 < NB:
                    next_rhs = issue_chunks(nb + 1, [4] * 8)
                for m in ms:
                    ot = out_pool.tile([P, NW], f32, tag="ot", name=f"ot_{nb}_{m}")
                    nc.vector.tensor_copy(out=ot[:, :], in_=psums[m][:, :])
                    nc.gpsimd.dma_start(
                        out=out[m * P:(m + 1) * P, ncols], in_=ot[:, :])


def _lib4_body(nc, tc, kxm, kxn, out, mybir):
    """Stock composable matmul schedule, except the LAST output tile
    (computed right before the exit barrier) evicts per-128-row on the
    idle vector engine and stores per-subtile, so its first bytes hit
    DRAM while the remaining psum banks are still draining."""
    from contextlib import ExitStack
    from concourse.kernels.tile_matmul import (
        composable_matmul_tile_kernel, dma_from_dram_kxm, dma_from_dram_kxn,
        dma_to_dram_mxn, k_pool_min_bufs, scalar_copyback)
    from concourse.bass import ds

    with ExitStack() as ctx:
        num_bufs = k_pool_min_bufs(kxn, max_tile_size=512)
        kxm_pool = ctx.enter_context(tc.tile_pool(name="kxm_pool", bufs=num_bufs))
        kxn_pool = ctx.enter_context(tc.tile_pool(name="kxn_pool", bufs=num_bufs))
        kxm_producer, kxm_shape = dma_from_dram_kxm(kxm_pool, kxm)
        kxn_producer, kxn_shape = dma_from_dram_kxn(kxn_pool, kxn)

        stock_consumer = dma_to_dram_mxn(out)
        stock_reducer = scalar_copyback()
        out_t = out.rearrange("(po pi) f -> pi po f", pi=128)
        LAST_M, LAST_N = M_CORE // 512 - 1, D_OUT // 512 - 1

        def is_last(md):
            return md.m_tile_idx == LAST_M and md.n_tile_idx == LAST_N

        def reducer(nc, psum, sbuf, md):
            if is_last(md):
                nc.vector.tensor_copy(out=sbuf, in_=psum)
            else:
                stock_reducer(nc, psum, sbuf, md)

        def consumer(nc, mxn_tile, md):
            if is_last(md):
                n_sl = ds(md.n_tile_idx * md.n_tile, md.n_slice_size)
                for s in range(md.m_subtiles):
                    nc.sync.dma_start(
                        out_t[:, ds(md.m_tile_idx * md.m_subtiles + s, 1), n_sl],
                        mxn_tile[:, ds(s, 1), :md.n_slice_size])
            else:
                stock_consumer(nc, mxn_tile, md)

        composable_matmul_tile_kernel(
            tc=tc,
            kxm_shape=kxm_shape,
            kxn_shape=kxn_shape,
            output_type=mybir.dt.float32,
            kxm_producer=kxm_producer,
            kxn_producer=kxn_producer,
            mxn_consumer=consumer,
            mxn_subtile_reducer=reducer)


def _build():
    """Build + compile the 8-core SPMD Bass program once per process."""
    if "nc" in _cache:
        return _cache["nc"]

    import concourse.bacc as bacc
    import concourse.tile as tile
    import concourse.mybir as mybir
    from concourse.kernels.tile_matmul import matmul_tile_kernel

    mm_dt = {"f32r": mybir.dt.float32r, "bf16": mybir.dt.bfloat16}[DTYPE]

    nc = bacc.Bacc("TRN2", target_bir_lowering=False, debug=False,
                   enable_asserts=bool(os.environ.get("BK_ASSERTS")), num_devices=NCORES)
    kxm = nc.dram_tensor("kxm", [D_IN, M_CORE], mm_dt,
                         kind="ExternalInput").ap()
    kxn = nc.dram_tensor("kxn", [D_IN, D_OUT], mm_dt,
                         kind="ExternalInput").ap()
    out = nc.dram_tensor("out", [M_CORE, D_OUT], mybir.dt.float32,
                         kind="ExternalOutput").ap()
    def _warmup(tc):
        # The PE clock is HAM-throttled to 1.2GHz until ~3.4us of
        # sustained matmul activity. The first real matmul can't start
        # until its DMAs land (~13us in), so burn that window warming
        # the clock gate with matmuls on memset tiles; their PSUM bank
        # frees on pool exit before the real kernel allocates.
        from contextlib import ExitStack
        with ExitStack() as ctx:
            wp = ctx.enter_context(tc.tile_pool(name="warm", bufs=1))
            wpp = ctx.enter_context(
                tc.tile_pool(name="warmp", bufs=1, space="PSUM"))
            wdt = mybir.dt.bfloat16
            a = wp.tile([128, 128], wdt)
            b = wp.tile([128, 512], wdt)
            nc.any.memset(a[:, :], 0.0)
            nc.any.memset(b[:, :], 0.0)
            ps = wpp.tile([128, 512], mybir.dt.float32)
            for _ in range(int(os.environ.get("BK_WARM", "12"))):
                nc.tensor.matmul(ps[:, :], lhsT=a[:, :], rhs=b[:, :],
                                 start=True, stop=True)

    if IMPL == "custom":
        with tile.TileContext(nc) as tc:
            _warmup(tc)
            _custom_body(nc, tc, kxm, kxn, out, mm_dt, mybir)
    elif IMPL == "lib4":
        with tile.TileContext(nc) as tc:
            _warmup(tc)
            _lib4_body(nc, tc, kxm, kxn, out, mybir)
    else:
        kw = {}
        if os.environ.get("BK_MAX_K_TILE"):
            kw["MAX_K_TILE_SIZE"] = int(os.environ["BK_MAX_K_TILE"])
        if os.environ.get("BK_SKIP_K_SNAKE"):
            kw["skip_k_snake"] = True
        if os.environ.get("BK_NO_CACHE_TILES"):
            kw["cache_tiles"] = False
        with tile.TileContext(nc) as tc:
            _warmup(tc)
            matmul_tile_kernel(tc, kxm, kxn, out, **kw)
    nc.compile()
    _cache["nc"] = nc
    return nc


def _prep_inputs(x, weight):
    if DTYPE == "bf16":
        import ml_dtypes
        np_dt = ml_dtypes.bfloat16
    else:
        np_dt = np.float32
    x2d = np.asarray(x, dtype=np.float32).reshape(M_TOTAL, D_IN)
    kxn = np.ascontiguousarray(np.sign(weight, dtype=np.float32).T.astype(np_dt))
    in_maps = []
    for c in range(NCORES):
        kxm = np.ascontiguousarray(x2d[c * M_CORE:(c + 1) * M_CORE].T.astype(np_dt))
        in_maps.append({"kxm": kxm, "kxn": kxn})
    return in_maps


def _run(x, weight, bias, trace=False):
    from concourse.bass_utils import run_bass_kernel_spmd

    nc = _build()
    in_maps = _prep_inputs(x, weight)
    res = run_bass_kernel_spmd(nc, in_maps, core_ids=list(range(NCORES)),
                               trace=trace)
    out = np.concatenate([res.results[c]["out"] for c in range(NCORES)],
                         axis=0)
    bias = np.asarray(bias, dtype=np.float32)
    if np.any(bias):
        out += bias
    return out.reshape(B, S, D_OUT), res


def kernel(x, weight, bias):
    out, _ = _run(x, weight, bias, trace=False)
    return out
